# revision 1
# baseline (speedup 1.0000x reference)
"""Trainium2 Bass kernel for ConvChebTemp (Chebyshev graph conv with temporal weights).

Math: out[b,v,o] = sum_{k,t,f} T_k(L)x0[:,t,f,b] w[f,k,t,o] + bias[o]
with x0 = inputs permuted to [V, T*Fin*B] and T_k the Chebyshev recurrence.

Key reformulation (Clenshaw): since the f/t contraction commutes with L,
contract weights FIRST: z_k[v, b, o] = sum_{t,f} x0[v,t,f,b] w[f,k,t,o], then
  b3 = z3; b2 = z2 + 2 L b3; b1 = z1 + 2 L b2 - b3; out = z0 + L b1 - b2 + bias
This shrinks every SpMM's column count 4x (256 -> 64 per batch).

Sharding: data-parallel over batch B=16 -> 2 batches per core, 8 cores.
Each SpMM is gather (dma_gather, sorted-by-row CSR, 512 B rows) + per-chunk
selection matmuls on the PE (selection matrices built on-chip in one DVE
tensor_scalar op from O(NNZ) metadata).
"""
import sys

sys.path.insert(0, "/opt/trn_rl_repo")

from contextlib import ExitStack  # noqa: E402

import numpy as np  # noqa: E402

from concourse import bacc, bass, mybir, tile  # noqa: E402
from concourse.bass_utils import run_bass_kernel_spmd  # noqa: E402

P = 128
N_CORES = 8
FP32 = mybir.dt.float32
I16 = mybir.dt.int16

# Problem dims (hardcoded per spec)
B, V, T, FIN = 16, 12288, 4, 64
KV, KT, FOUT = 4, 4, 64
BC = B // N_CORES          # batches per core
F = BC * FOUT              # spmm column width per core (both batches interleaved)
C = T * FIN                # z-matmul contraction dim
GBUF_BUFS = 6
PSZ_BUFS = 2
PST_BUFS = 2
PSS_BUFS = 4


def _preprocess_lap(lap_rows, lap_cols, lap_vals, v):
    """Sort nnz by row, pad each 128-row out-tile's run to a multiple of P.

    Returns (gidx [16, NNZP//16] int16 wrapped, growl [P, NCHUNK] f32,
    gval [P, NCHUNK] f32, chunks_per_tile list).
    """
    nt = v // P
    order = np.argsort(lap_rows, kind="stable")
    srows = lap_rows[order]
    scols = lap_cols[order]
    svals = lap_vals[order]
    tile_of = srows // P
    # counts per tile
    counts = np.bincount(tile_of, minlength=nt)
    chunks_per_tile = [max(1, int(-(-c // P))) for c in counts]
    nnzp = sum(chunks_per_tile) * P
    gidx = np.zeros(nnzp, np.int16)
    growl = np.zeros(nnzp, np.float32)
    gval = np.zeros(nnzp, np.float32)
    # fill per tile
    starts = np.zeros(nt + 1, np.int64)
    np.cumsum(counts, out=starts[1:])
    pos = 0
    for t in range(nt):
        n = int(counts[t])
        s = int(starts[t])
        gidx[pos:pos + n] = scols[s:s + n]
        growl[pos:pos + n] = (srows[s:s + n] - t * P).astype(np.float32)
        gval[pos:pos + n] = svals[s:s + n]
        # padding slots: col 0, rowl 0, val 0 (contribute nothing)
        pos += chunks_per_tile[t] * P
    assert pos == nnzp
    nchunk = nnzp // P
    # wrapped int16 layout for dma_gather: slot s -> [s % 16, s // 16]
    gidx_w = gidx.reshape(-1, 16).T.copy()          # [16, NNZP//16]
    gidx_w = np.tile(gidx_w, (8, 1))                # replicate for 8 q7 cores
    growl_m = growl.reshape(nchunk, P).T.copy()     # [P, NCHUNK]
    gval_m = gval.reshape(nchunk, P).T.copy()       # [P, NCHUNK]
    return gidx_w, growl_m, gval_m, chunks_per_tile


def build_program(v, chunks_per_tile, n_cores=N_CORES, max_phase=3):
    """Build the SPMD Bass program (identical across cores)."""
    nt = v // P
    nchunk = sum(chunks_per_tile)
    nnzp = nchunk * P
    nc = bacc.Bacc("TRN2", target_bir_lowering=False, debug=False,
                   num_devices=n_cores)

    xin = nc.dram_tensor("xin", [BC, v, T, FIN], FP32, kind="ExternalInput")
    wz = nc.dram_tensor("wz", [P, 2 * KV * FOUT], FP32, kind="ExternalInput")
    bias_d = nc.dram_tensor("bias128", [P, F], FP32, kind="ExternalInput")
    iota_d = nc.dram_tensor("iota128", [P, P], FP32, kind="ExternalInput")
    ident_d = nc.dram_tensor("ident128", [P, P], FP32, kind="ExternalInput")
    gidx_d = nc.dram_tensor("gidx", [P, nnzp // 16], I16, kind="ExternalInput")
    growl_d = nc.dram_tensor("growl", [P, nchunk], FP32, kind="ExternalInput")
    gval1_d = nc.dram_tensor("gval1", [P, nchunk], FP32, kind="ExternalInput")
    gval2_d = nc.dram_tensor("gval2", [P, nchunk], FP32, kind="ExternalInput")
    out_d = nc.dram_tensor("out", [BC, v, FOUT], FP32, kind="ExternalOutput")

    with tile.TileContext(nc) as tc, ExitStack() as ctx:
        dram = ctx.enter_context(tc.tile_pool(name="dram", bufs=1, space="DRAM"))
        z0_d = dram.tile([v, F], FP32, tag="z0d")
        b3_d = dram.tile([v, F], FP32, tag="b3d")
        b2_d = dram.tile([v, F], FP32, tag="b2d")
        b1_d = dram.tile([v, F], FP32, tag="b1d")

        const = ctx.enter_context(tc.tile_pool(name="const", bufs=1))
        res = ctx.enter_context(tc.tile_pool(name="res", bufs=1))
        xpool = ctx.enter_context(tc.tile_pool(name="x", bufs=3))
        xtp = ctx.enter_context(tc.tile_pool(name="xt", bufs=3))
        stg = ctx.enter_context(tc.tile_pool(name="stg", bufs=3))
        gpool = ctx.enter_context(tc.tile_pool(name="gbuf", bufs=GBUF_BUFS))
        spool = ctx.enter_context(tc.tile_pool(name="sel", bufs=4))
        bpool = ctx.enter_context(tc.tile_pool(name="bt", bufs=3))
        tpool = ctx.enter_context(tc.tile_pool(name="tmp", bufs=3))
        psz = ctx.enter_context(tc.tile_pool(name="psz", bufs=PSZ_BUFS, space="PSUM"))
        pst = ctx.enter_context(tc.tile_pool(name="pst", bufs=PST_BUFS, space="PSUM"))
        pss = ctx.enter_context(tc.tile_pool(name="pss", bufs=PSS_BUFS, space="PSUM"))

        # constants + metadata resident in SBUF
        iota_sb = const.tile([P, P], FP32, tag="iota")
        nc.sync.dma_start(iota_sb[:], iota_d[:, :])
        ident_sb = const.tile([P, P], FP32, tag="ident")
        nc.sync.dma_start(ident_sb[:], ident_d[:, :])
        bias_sb = const.tile([P, F], FP32, tag="bias")
        nc.sync.dma_start(bias_sb[:], bias_d[:, :])
        wz_sb = const.tile([P, 2 * KV * FOUT], FP32, tag="wz")
        nc.sync.dma_start(wz_sb[:], wz[:, :])
        gidx_sb = const.tile([P, nnzp // 16], I16, tag="gidx")
        nc.sync.dma_start(gidx_sb[:], gidx_d[:, :])
        growl_sb = const.tile([P, nchunk], FP32, tag="growl")
        nc.sync.dma_start(growl_sb[:], growl_d[:, :])
        gval1_sb = const.tile([P, nchunk], FP32, tag="gval1")
        nc.sync.dma_start(gval1_sb[:], gval1_d[:, :])
        gval2_sb = const.tile([P, nchunk], FP32, tag="gval2")
        nc.sync.dma_start(gval2_sb[:], gval2_d[:, :])

        # per-vt 256-col block: [z1_b0 | z2_b0 | z1_b1 | z2_b1]
        z12_res = res.tile([P, nt * 2 * F], FP32, tag="z12")
        z12v = z12_res[:].rearrange("p (t x o) -> p t x o", x=4, o=FOUT)

        # ---------- phase Z: z_k = x0 @ w_k for all k ----------
        for vt in range(nt):
            v0 = vt * P
            # stage layout: [z0_b0 | z3_b0 | z0_b1 | z3_b1]
            st = stg.tile([P, 2 * F], FP32, tag="st")
            stv = st[:].rearrange("p (x o) -> p x o", o=FOUT)
            for b in range(BC):
                xt = xpool.tile([P, C], FP32, tag="xnat")
                nc.sync.dma_start(
                    xt[:], xin[b, v0:v0 + P, :, :].rearrange("p t f -> p (t f)"))
                tps = pst.tile([P, C], FP32, tag="tps")
                for cc in range(2):
                    nc.tensor.matmul(tps[:, cc * P:(cc + 1) * P],
                                     lhsT=xt[:, cc * P:(cc + 1) * P],
                                     rhs=ident_sb[:], is_transpose=True,
                                     start=True, stop=True)
                xT2 = xtp.tile([P, C], FP32, tag="xT")
                nc.vector.tensor_copy(xT2[:], tps[:])
                zps = psz.tile([P, KV * FOUT], FP32, tag="zps")
                for cc in range(2):
                    nc.tensor.matmul(zps[:], lhsT=xT2[:, cc * P:(cc + 1) * P],
                                     rhs=wz_sb[:, cc * KV * FOUT:(cc + 1) * KV * FOUT],
                                     start=(cc == 0), stop=(cc == 1))
                # zps cols = [z0 | z3 | z1 | z2] for this b
                nc.vector.tensor_copy(st[:, b * F:(b + 1) * F], zps[:, 0:F])
                nc.vector.tensor_copy(z12_res[:, vt * 2 * F + b * F:
                                              vt * 2 * F + (b + 1) * F],
                                      zps[:, F:2 * F])
            nc.sync.dma_start(
                z0_d[v0:v0 + P, :].rearrange("p (x o) -> p x o", o=FOUT),
                stv[:, 0::2, :])
            nc.sync.dma_start(
                b3_d[v0:v0 + P, :].rearrange("p (x o) -> p x o", o=FOUT),
                stv[:, 1::2, :])

        # ---------- spmm phases ----------
        # dma_gather is capped at 1024 indices per instruction (the SWDGE
        # descriptor ring holds 16 rings x 64 descs); gather in 8-chunk pieces
        # that may span out-tile boundaries.
        CHUNKS_PER_PIECE = 8

        def spmm_phase(src_d, vals_sb, combine):
            state = {"gb": None, "base": 0, "len": 0}

            def ensure_piece(c):
                while state["gb"] is None or c >= state["base"] + state["len"]:
                    base = 0 if state["gb"] is None else state["base"] + state["len"]
                    plen = min(CHUNKS_PER_PIECE, nchunk - base)
                    gb = gpool.tile([P, plen, P], FP32, tag="gb")
                    s0 = base * P
                    nidx = plen * P
                    nc.gpsimd.dma_gather(
                        out_ap=gb[:],
                        in_ap=src_d[:, :],
                        idxs_ap=gidx_sb[:, s0 // 16:(s0 + nidx) // 16],
                        num_idxs=nidx,
                        num_idxs_reg=nidx,
                        elem_size=F,
                    )
                    state.update(gb=gb, base=base, len=plen)
                return state["gb"], state["base"]

            ci = 0
            for tt in range(nt):
                nck = chunks_per_tile[tt]
                ps = pss.tile([P, F], FP32, tag="ps")
                for k in range(nck):
                    col = ci + k
                    gb, base = ensure_piece(col)
                    sT = spool.tile([P, P], FP32, tag="sT")
                    nc.vector.tensor_scalar(
                        out=sT[:], in0=iota_sb[:],
                        scalar1=growl_sb[:, col:col + 1],
                        scalar2=vals_sb[:, col:col + 1],
                        op0=mybir.AluOpType.is_equal,
                        op1=mybir.AluOpType.mult,
                    )
                    nc.tensor.matmul(ps[:], lhsT=sT[:], rhs=gb[:, col - base, :],
                                     start=(k == 0), stop=(k == nck - 1))
                combine(tt, ps)
                ci += nck

        def ps3(ps):
            return ps[:].rearrange("p (x o) -> p x o", o=FOUT)

        def dram3(d, tt):
            return d[tt * P:(tt + 1) * P, :].rearrange("p (x o) -> p x o", o=FOUT)

        # spmm 1: b2 = z2 + 2 L b3   (z2 slots become b2 in place)
        def combine1(tt, ps):
            zsl = z12v[:, tt, 1::2, :]
            nc.vector.tensor_tensor(out=zsl, in0=ps3(ps), in1=zsl,
                                    op=mybir.AluOpType.add)
            nc.sync.dma_start(dram3(b2_d, tt), zsl)

        if max_phase >= 1:
            spmm_phase(b3_d, gval2_sb, combine1)

        # spmm 2: b1 = z1 + 2 L b2 - b3
        def combine2(tt, ps):
            zsl = z12v[:, tt, 0::2, :]
            b3t = bpool.tile([P, F], FP32, tag="b3t")
            nc.sync.dma_start(b3t[:], b3_d[tt * P:(tt + 1) * P, :])
            tmp = tpool.tile([P, F], FP32, tag="tmp")
            nc.vector.tensor_tensor(out=ps3(tmp), in0=ps3(ps), in1=zsl,
                                    op=mybir.AluOpType.add)
            nc.vector.tensor_tensor(out=tmp[:], in0=tmp[:], in1=b3t[:],
                                    op=mybir.AluOpType.subtract)
            nc.sync.dma_start(b1_d[tt * P:(tt + 1) * P, :], tmp[:])

        if max_phase >= 2:
            spmm_phase(b2_d, gval2_sb, combine2)

        # spmm 3: out = z0 + L b1 - b2 + bias
        def combine3(tt, ps):
            b2sl = z12v[:, tt, 1::2, :]
            z0t = bpool.tile([P, F], FP32, tag="z0t")
            nc.sync.dma_start(z0t[:], z0_d[tt * P:(tt + 1) * P, :])
            tmp = tpool.tile([P, F], FP32, tag="otmp")
            nc.vector.tensor_tensor(out=ps3(tmp), in0=ps3(ps), in1=b2sl,
                                    op=mybir.AluOpType.subtract)
            nc.vector.tensor_tensor(out=tmp[:], in0=tmp[:], in1=z0t[:],
                                    op=mybir.AluOpType.add)
            nc.vector.tensor_tensor(out=tmp[:], in0=tmp[:], in1=bias_sb[:],
                                    op=mybir.AluOpType.add)
            for b in range(BC):
                nc.sync.dma_start(out_d[b, tt * P:(tt + 1) * P, :],
                                  tmp[:, b * FOUT:(b + 1) * FOUT])

        if max_phase >= 3:
            spmm_phase(b1_d, gval1_sb, combine3)

    nc.compile()
    return nc


def make_host_inputs(inputs, weight, bias, lap_vals, lap_rows, lap_cols, v=V):
    """Build the per-core input maps + preprocessing. Returns (in_maps, chunks)."""
    gidx_w, growl_m, gval_m, chunks = _preprocess_lap(
        np.asarray(lap_rows), np.asarray(lap_cols),
        np.asarray(lap_vals, np.float32), v)
    w = np.asarray(weight, np.float32)
    # wz[cc, c_local, k*FOUT+o] where c = t*FIN+f = cc*128+c_local
    w = w[:, [0, 3, 1, 2], :, :]  # k order [z0, z3, z1, z2]
    wz = np.transpose(w, (2, 0, 1, 3)).reshape(C, KV * FOUT)  # [(t f), (k o)]
    # [c, ko] -> [c_local, cc*256 + ko]
    wz = np.ascontiguousarray(
        wz.reshape(2, P, KV * FOUT).transpose(1, 0, 2).reshape(P, 2 * KV * FOUT))
    bias128 = np.ascontiguousarray(
        np.tile(np.asarray(bias, np.float32), (P, BC)))
    iota128 = np.ascontiguousarray(
        np.broadcast_to(np.arange(P, dtype=np.float32)[None, :], (P, P)))
    ident128 = np.eye(P, dtype=np.float32)
    common = {
        "wz": wz,
        "bias128": bias128,
        "iota128": iota128,
        "ident128": ident128,
        "gidx": np.ascontiguousarray(gidx_w),
        "growl": np.ascontiguousarray(growl_m),
        "gval1": np.ascontiguousarray(gval_m),
        "gval2": np.ascontiguousarray(2.0 * gval_m),
    }
    xin = np.asarray(inputs, np.float32)
    in_maps = []
    for r in range(N_CORES):
        m = dict(common)
        m["xin"] = np.ascontiguousarray(xin[BC * r:BC * (r + 1)])
        in_maps.append(m)
    return in_maps, chunks


_CACHE = {}


def _get_program(chunks):
    key = tuple(chunks)
    if key not in _CACHE:
        _CACHE[key] = build_program(V, list(chunks))
    return _CACHE[key]


def kernel(inputs, weight, bias, lap_vals, lap_rows, lap_cols):
    in_maps, chunks = make_host_inputs(inputs, weight, bias, lap_vals,
                                       lap_rows, lap_cols)
    nc = _get_program(chunks)
    res = run_bass_kernel_spmd(nc, in_maps, list(range(N_CORES)))
    out = np.concatenate([res.results[r]["out"] for r in range(N_CORES)], axis=0)
    return np.ascontiguousarray(out.astype(np.float32))


def time_kernel(inputs_dict, iters=3):
    """Wall-clock repeated executions of the cached program (ns per run)."""
    import time

    in_maps, chunks = make_host_inputs(**inputs_dict)
    nc = _get_program(chunks)
    times = []
    for _ in range(iters):
        t0 = time.perf_counter()
        run_bass_kernel_spmd(nc, in_maps, list(range(N_CORES)))
        times.append(time.perf_counter() - t0)
    return min(times) * 1e9



# revision 15
# speedup vs baseline: 1.4977x; 1.4977x over previous
"""Trainium2 Bass kernel for ConvChebTemp (Chebyshev graph conv, temporal weights).

Math: out[b,v,o] = sum_{k,t,f} T_k(L)x0[:,t,f,b] w[f,k,t,o] + bias[o]
with x0 = inputs permuted to [V, T*Fin*B] and T_k the Chebyshev recurrence.

Clenshaw reformulation (weights contracted first):
  z_k[v, b, o] = sum_{t,f} x0[v,t,f,b] w[f,k,t,o]
  b3 = z3; b2 = z2 + 2 L b3; b1 = z1 + 2 L b2 - b3; out = z0 + L b1 - b2 + bias

Design (one NeuronCore per 2 batches, 8 cores data-parallel over B=16):
- Host pre-transposes x to [c, v] layout in bf16, so the z matmuls need no
  on-chip transposes: zps[v, (k,b,o)] accumulates directly in PSUM with a
  column layout [z0|z1|z2|z3] (each 128 = [b0 64|b1 64]).
- z0..z3 kept SBUF-resident in bf16 (zres); b3/b2/b1 gather sources in DRAM
  bf16 (256 B rows).
- SpMM via dma_gather (sorted-by-row, exact-NNZ chunks that may span
  out-tile boundaries; spanning chunks get one selection matmul per tile,
  with out-of-tile lanes masked by rowl=-1) + bf16 selection matmuls.
- Combine terms (z adds/subs, bias) are folded into the PSUM accumulation as
  identity matmuls on the otherwise-idle PE; PSUM->SBUF eviction on the ACT
  engine. DVE only builds selection matrices.
"""
import sys

sys.path.insert(0, "/opt/trn_rl_repo")

from contextlib import ExitStack  # noqa: E402

import numpy as np  # noqa: E402

from concourse import bacc, bass, mybir, tile  # noqa: E402
from concourse.bass_utils import run_bass_kernel_spmd  # noqa: E402

try:
    import ml_dtypes  # noqa: E402

    BF16_NP = ml_dtypes.bfloat16
except ImportError:  # pragma: no cover
    import jax.numpy as jnp  # noqa: E402

    BF16_NP = jnp.bfloat16

P = 128
N_CORES = 8
FP32 = mybir.dt.float32
BF16 = mybir.dt.bfloat16
I16 = mybir.dt.int16

# Problem dims (hardcoded per spec)
B, V, T, FIN = 16, 12288, 4, 64
KV, KT, FOUT = 4, 4, 64
BC = B // N_CORES          # batches per core
F = BC * FOUT              # spmm column width per core (both batches)
C = T * FIN                # z-matmul contraction dim (= 2 chunks of 128)
NT = V // P                # out tiles
NNZ = 9 * V
CH = NNZ // P              # 128-nnz chunks, exact (no padding)
PIECE = 1024               # gather idxs per dma_gather (HW SWDGE ring cap)
PSZ_BUFS = 4
PSS_BUFS = 4
SPOOL_BUFS = 8
GPOOL_BUFS = 8
N_PIECES = NNZ // PIECE
SPAN = 1536                # xT columns loaded per DMA in the z phase
ZB = 4 * F                 # zps column count: [z0|z1|z2|z3]


def _preprocess_lap(lap_rows, lap_cols, lap_vals):
    """Sort nnz by row; emit per-(chunk, tile) selection-matmul metadata.

    Chunks are exact 128-nnz groups of the row-sorted nonzeros. A chunk whose
    rows cross an out-tile boundary yields one matmul entry per tile touched;
    lanes outside the entry's tile are masked with rowl = -1.

    Returns (gidx [128, NNZ//16] int16, growl/gval1/gval2 [128, NMM] f32,
    mm_chunk [NMM], mm_tile [NMM]).
    """
    assert lap_rows.shape == (NNZ,)
    order = np.argsort(lap_rows, kind="stable")
    srows = lap_rows[order].astype(np.int64)
    scols = lap_cols[order].astype(np.int64)
    svals = lap_vals[order].astype(np.float32)
    tile_of = srows // P

    mm_chunk, mm_tile = [], []
    growl_cols, gval_cols = [], []
    for k in range(CH):
        s0 = k * P
        tl = tile_of[s0:s0 + P]
        rows = srows[s0:s0 + P]
        vals = svals[s0:s0 + P]
        for t in np.unique(tl):
            mask = tl == t
            rowl = np.where(mask, rows - t * P, -1).astype(np.float32)
            mm_chunk.append(k)
            mm_tile.append(int(t))
            growl_cols.append(rowl)
            gval_cols.append(vals)
    nmm = len(mm_chunk)
    growl = np.stack(growl_cols, axis=1)          # [128, NMM]
    gval = np.stack(gval_cols, axis=1)            # [128, NMM]

    gidx = scols.astype(np.int16).reshape(-1, 16).T.copy()   # [16, NNZ//16]
    gidx = np.tile(gidx, (8, 1))                             # [128, NNZ//16]
    return (np.ascontiguousarray(gidx), np.ascontiguousarray(growl),
            np.ascontiguousarray(gval), np.ascontiguousarray(2.0 * gval),
            tuple(mm_chunk), tuple(mm_tile))


def build_program(mm_chunk, mm_tile, n_cores=N_CORES, max_phase=3):
    """Build the SPMD Bass program (identical across cores)."""
    nmm = len(mm_chunk)
    # per-tile contiguous mm ranges
    tile_mms = [[] for _ in range(NT)]
    for mi, t in enumerate(mm_tile):
        tile_mms[t].append(mi)

    nc = bacc.Bacc("TRN2", target_bir_lowering=False, debug=False,
                   num_devices=n_cores, dynamic_dma_scratch_size=16384)

    xt_d = nc.dram_tensor("xt", [BC, 2, P, V], BF16, kind="ExternalInput")
    wz_d = nc.dram_tensor("wz", [P, 2 * KV * FOUT], BF16, kind="ExternalInput")
    ident_d = nc.dram_tensor("ident", [P, P], BF16, kind="ExternalInput")
    nident_d = nc.dram_tensor("nident", [P, P], BF16, kind="ExternalInput")
    ones_d = nc.dram_tensor("onescol", [P, P], BF16, kind="ExternalInput")
    biasm_d = nc.dram_tensor("biasmat", [P, F], BF16, kind="ExternalInput")
    iota_d = nc.dram_tensor("iota", [P, P], BF16, kind="ExternalInput")
    gidx_d = nc.dram_tensor("gidx", [P, NNZ // 16], I16, kind="ExternalInput")
    growl_d = nc.dram_tensor("growl", [P, nmm], FP32, kind="ExternalInput")
    gval1_d = nc.dram_tensor("gval1", [P, nmm], FP32, kind="ExternalInput")
    gval2_d = nc.dram_tensor("gval2", [P, nmm], FP32, kind="ExternalInput")
    out_d = nc.dram_tensor("out", [V, F], FP32, kind="ExternalOutput")

    with tile.TileContext(nc) as tc, ExitStack() as ctx:
        dram = ctx.enter_context(tc.tile_pool(name="dram", bufs=1, space="DRAM"))
        b3_d = dram.tile([V, F], BF16, tag="b3d")
        b2_d = dram.tile([V, F], BF16, tag="b2d")
        b1_d = dram.tile([V, F], BF16, tag="b1d")

        const = ctx.enter_context(tc.tile_pool(name="const", bufs=1))
        xsp = ctx.enter_context(tc.tile_pool(name="xsp", bufs=2))
        zres_p = ctx.enter_context(tc.tile_pool(name="zresp", bufs=1))
        gpool = ctx.enter_context(tc.tile_pool(name="gbuf", bufs=GPOOL_BUFS))
        spool = ctx.enter_context(tc.tile_pool(name="sel", bufs=SPOOL_BUFS))
        stg = ctx.enter_context(tc.tile_pool(name="stg", bufs=3))
        ostg = ctx.enter_context(tc.tile_pool(name="ostg", bufs=3))
        psz = ctx.enter_context(tc.tile_pool(name="psz", bufs=PSZ_BUFS, space="PSUM"))
        pss = ctx.enter_context(tc.tile_pool(name="pss", bufs=PSS_BUFS, space="PSUM"))

        def cload(shape, dtype, src, tag):
            t = const.tile(shape, dtype, tag=tag)
            nc.sync.dma_start(t[:], src[:, :])
            return t

        wz_sb = cload([P, 2 * KV * FOUT], BF16, wz_d, "wz")
        ident_sb = cload([P, P], BF16, ident_d, "ident")
        nident_sb = cload([P, P], BF16, nident_d, "nident")
        ones_sb = cload([P, P], BF16, ones_d, "ones")
        biasm_sb = cload([P, F], BF16, biasm_d, "biasm")
        iota_sb = cload([P, P], BF16, iota_d, "iota")
        # big spmm metadata is loaded after the first x spans (see below) so
        # the Z phase's critical path starts immediately
        meta_sb = {}

        def load_meta():
            meta_sb["gidx"] = cload([P, NNZ // 16], I16, gidx_d, "gidx")
            meta_sb["growl"] = cload([P, nmm], FP32, growl_d, "growl")
            meta_sb["gval1"] = cload([P, nmm], FP32, gval1_d, "gval1")
            meta_sb["gval2"] = cload([P, nmm], FP32, gval2_d, "gval2")

        # z0..z3 resident, bf16: per tile 4 blocks of F columns [z0|z1|z2|z3]
        zres = zres_p.tile([P, NT * ZB], BF16, tag="zres")

        def zslot(vt, j):
            return zres[:, vt * ZB + j * F:vt * ZB + (j + 1) * F]

        zresv = zres[:].rearrange("p (t z) -> p t z", z=ZB)

        def zsrc_group(vt0, n, j):
            """[p, n, F] strided view of z-block j for tiles vt0..vt0+n."""
            return zresv[:, vt0:vt0 + n, j * F:(j + 1) * F]

        def dram_group(d, vt0, n, dt=None):
            """[p, n, F] view of DRAM rows vt0*P..(vt0+n)*P."""
            return d[vt0 * P:(vt0 + n) * P, :].rearrange(
                "(t p) f -> p t f", p=P)

        # ---------- Z phase: z_k = x0 @ w_k, all k, no transposes ----------
        n_spans = V // SPAN
        vt_per_span = SPAN // P
        GRP = 8
        for si in range(n_spans):
            sp = {}
            for b in range(BC):
                xt = xsp.tile([P, 2, SPAN], BF16, tag=f"x{b}")
                nc.sync.dma_start(
                    xt[:], xt_d[b, :, :, si * SPAN:(si + 1) * SPAN].rearrange(
                        "c p v -> p c v"))
                sp[b] = xt
            if si == 1:
                load_meta()
            for vj in range(vt_per_span):
                vt = si * vt_per_span + vj
                off = vj * P
                zps = psz.tile([P, ZB], FP32, tag="zps")
                for bb in range(BC):
                    for k in range(KV):
                        dst = zps[:, k * F + bb * FOUT:k * F + (bb + 1) * FOUT]
                        for cc in range(2):
                            nc.tensor.matmul(
                                dst, lhsT=sp[bb][:, cc, off:off + P],
                                rhs=wz_sb[:, (cc * KV + k) * FOUT:
                                          (cc * KV + k + 1) * FOUT],
                                start=(cc == 0), stop=(cc == 1))
                # evict psum -> zres (DVE lower half, ACT upper half)
                nc.vector.tensor_copy(zres[:, vt * ZB:vt * ZB + 2 * F],
                                      zps[:, 0:2 * F])
                nc.scalar.copy(out=zres[:, vt * ZB + 2 * F:vt * ZB + ZB],
                               in_=zps[:, 2 * F:ZB])
                # b3 = z3 to DRAM (gather source), 8 tiles per DMA
                if vt % GRP == GRP - 1:
                    nc.sync.dma_start(dram_group(b3_d, vt - GRP + 1, GRP),
                                      zsrc_group(vt - GRP + 1, GRP, 3))

        # ---------- spmm phases ----------
        def spmm_phase(src_d, vals_sb, extras, sink):
            state = {"gb": None, "base": -1}

            def ensure_piece(k):
                pc = (k * P) // PIECE
                if state["base"] != pc:
                    gb = gpool.tile([P, PIECE // P, F], BF16, tag="gb")
                    s0 = pc * PIECE
                    nc.gpsimd.dma_gather(
                        out_ap=gb[:],
                        in_ap=src_d[:, :],
                        idxs_ap=gidx_sb[:, s0 // 16:(s0 + PIECE) // 16],
                        num_idxs=PIECE,
                        num_idxs_reg=PIECE,
                        elem_size=F,
                    )
                    state.update(gb=gb, base=pc)
                return state["gb"], pc

            for tt in range(NT):
                mms = tile_mms[tt]
                ps = pss.tile([P, F], FP32, tag="ps")
                for j, mi in enumerate(mms):
                    k = mm_chunk[mi]
                    gb, pc = ensure_piece(k)
                    sT = spool.tile([P, P], BF16, tag="sT")
                    nc.vector.tensor_scalar(
                        out=sT[:], in0=iota_sb[:],
                        scalar1=growl_sb[:, mi:mi + 1],
                        scalar2=vals_sb[:, mi:mi + 1],
                        op0=mybir.AluOpType.is_equal,
                        op1=mybir.AluOpType.mult,
                    )
                    nc.tensor.matmul(ps[:], lhsT=sT[:],
                                     rhs=gb[:, k - pc * (PIECE // P), :],
                                     start=(j == 0), stop=False)
                for e, (lhs_sb, rhs_of) in enumerate(extras):
                    nc.tensor.matmul(ps[:], lhsT=lhs_sb[:], rhs=rhs_of(tt),
                                     start=(not mms and e == 0),
                                     stop=(e == len(extras) - 1))
                sink(tt, ps)

        # phase 1: b2 = z2 + 2 L b3   (z2 slot becomes b2 in place)
        def sink1(tt, ps):
            nc.scalar.copy(out=zslot(tt, 2), in_=ps[:])
            if tt % GRP == GRP - 1:
                nc.sync.dma_start(dram_group(b2_d, tt - GRP + 1, GRP),
                                  zsrc_group(tt - GRP + 1, GRP, 2))

        gidx_sb = meta_sb["gidx"]
        gval1_sb = meta_sb["gval1"]
        gval2_sb = meta_sb["gval2"]
        growl_sb = meta_sb["growl"]

        if max_phase >= 1:
            spmm_phase(b3_d, gval2_sb,
                       [(ident_sb, lambda tt: zslot(tt, 2))], sink1)

        # phase 2: b1 = z1 + 2 L b2 - b3(=z3)
        st2 = {"t": None}

        def sink2(tt, ps):
            if tt % GRP == 0:
                b1st = stg.tile([P, GRP, F], BF16, tag="b1st")
                st2["t"] = b1st
            nc.scalar.copy(out=st2["t"][:, tt % GRP, :], in_=ps[:])
            if tt % GRP == GRP - 1:
                nc.sync.dma_start(dram_group(b1_d, tt - GRP + 1, GRP),
                                  st2["t"][:])

        if max_phase >= 2:
            spmm_phase(b2_d, gval2_sb,
                       [(ident_sb, lambda tt: zslot(tt, 1)),
                        (nident_sb, lambda tt: zslot(tt, 3))], sink2)

        # phase 3: out = z0 + L b1 - b2(=z2 slot) + bias
        st3 = {"t": None}

        def sink3(tt, ps):
            if tt % GRP == 0:
                ost = ostg.tile([P, GRP, F], FP32, tag="ost")
                st3["t"] = ost
            nc.scalar.copy(out=st3["t"][:, tt % GRP, :], in_=ps[:])
            if tt % GRP == GRP - 1:
                nc.sync.dma_start(dram_group(out_d, tt - GRP + 1, GRP),
                                  st3["t"][:])

        if max_phase >= 3:
            spmm_phase(b1_d, gval1_sb,
                       [(ident_sb, lambda tt: zslot(tt, 0)),
                        (nident_sb, lambda tt: zslot(tt, 2)),
                        (ones_sb, lambda tt: biasm_sb[:])], sink3)

    nc.compile()
    return nc


def make_host_inputs(inputs, weight, bias, lap_vals, lap_rows, lap_cols):
    """Build per-core input maps + metadata. Returns (in_maps, key)."""
    gidx, growl, gval1, gval2, mm_chunk, mm_tile = _preprocess_lap(
        np.asarray(lap_rows), np.asarray(lap_cols),
        np.asarray(lap_vals, np.float32))

    w = np.asarray(weight, np.float32)          # [Fin, Kv, Kt, Fout]
    # wz[c=(t,f)][k, o]; column layout per cc-chunk of c: (cc*KV + k)*FOUT + o
    wzc = np.transpose(w, (2, 0, 1, 3)).reshape(C, KV * FOUT)
    wz = np.ascontiguousarray(
        wzc.reshape(2, P, KV * FOUT).transpose(1, 0, 2).reshape(P, 2 * KV * FOUT)
    ).astype(BF16_NP)

    ident = np.eye(P, dtype=np.float32).astype(BF16_NP)
    nident = (-np.eye(P, dtype=np.float32)).astype(BF16_NP)
    ones_col = np.zeros((P, P), np.float32)
    ones_col[0, :] = 1.0
    ones_col = ones_col.astype(BF16_NP)
    biasm = np.zeros((P, F), np.float32)
    biasm[0, :] = np.tile(np.asarray(bias, np.float32), BC)
    biasm = biasm.astype(BF16_NP)
    iota = np.ascontiguousarray(
        np.broadcast_to(np.arange(P, dtype=np.float32)[None, :], (P, P))
    ).astype(BF16_NP)

    common = {
        "wz": wz, "ident": ident, "nident": nident, "onescol": ones_col,
        "biasmat": biasm, "iota": iota, "gidx": gidx, "growl": growl,
        "gval1": gval1, "gval2": gval2,
    }
    x = np.asarray(inputs, np.float32)          # [B, V, T, Fin]
    in_maps = []
    for r in range(N_CORES):
        xc = x[BC * r:BC * (r + 1)].reshape(BC, V, C)        # [BC, V, 256]
        xt = np.ascontiguousarray(
            xc.transpose(0, 2, 1).reshape(BC, 2, P, V)).astype(BF16_NP)
        m = dict(common)
        m["xt"] = xt
        in_maps.append(m)
    return in_maps, (mm_chunk, mm_tile)


_CACHE = {}


def _get_program(key):
    if key not in _CACHE:
        _CACHE[key] = build_program(key[0], key[1])
    return _CACHE[key]


def kernel(inputs, weight, bias, lap_vals, lap_rows, lap_cols):
    in_maps, key = make_host_inputs(inputs, weight, bias, lap_vals,
                                    lap_rows, lap_cols)
    nc = _get_program(key)
    res = run_bass_kernel_spmd(nc, in_maps, list(range(N_CORES)))
    outs = []
    for r in range(N_CORES):
        o = res.results[r]["out"]                # [V, BC*FOUT]
        outs.append(o.reshape(V, BC, FOUT).transpose(1, 0, 2))
    out = np.concatenate(outs, axis=0)
    return np.ascontiguousarray(out.astype(np.float32))


def time_kernel(inputs_dict, iters=3):
    """Wall-clock repeated executions of the cached program (ns per run)."""
    import time

    in_maps, key = make_host_inputs(**inputs_dict)
    nc = _get_program(key)
    times = []
    for _ in range(iters):
        t0 = time.perf_counter()
        run_bass_kernel_spmd(nc, in_maps, list(range(N_CORES)))
        times.append(time.perf_counter() - t0)
    return min(times) * 1e9


# revision 21
# speedup vs baseline: 1.5166x; 1.0126x over previous
"""Trainium2 Bass kernel for ConvChebTemp (Chebyshev graph conv, temporal weights).

Math: out[b,v,o] = sum_{k,t,f} T_k(L)x0[:,t,f,b] w[f,k,t,o] + bias[o]
with x0 = inputs permuted to [V, T*Fin*B] and T_k the Chebyshev recurrence.

Clenshaw reformulation (weights contracted first):
  z_k[v, b, o] = sum_{t,f} x0[v,t,f,b] w[f,k,t,o]
  b3 = z3; b2 = z2 + 2 L b3; b1 = z1 + 2 L b2 - b3; out = z0 + L b1 - b2 + bias

Design (one NeuronCore per 2 batches, 8 cores data-parallel over B=16):
- Host pre-transposes x to [c, v] layout in bf16, so the z matmuls need no
  on-chip transposes: zps[v, (k,b,o)] accumulates directly in PSUM with a
  column layout [z0|z1|z2|z3] (each 128 = [b0 64|b1 64]).
- z0..z3 kept SBUF-resident in bf16 (zres); b3/b2/b1 gather sources in DRAM
  bf16 (256 B rows).
- SpMM via dma_gather (sorted-by-row, exact-NNZ chunks that may span
  out-tile boundaries; spanning chunks get one selection matmul per tile,
  with out-of-tile lanes masked by rowl=-1) + bf16 selection matmuls.
- Combine terms (z adds/subs, bias) are folded into the PSUM accumulation as
  identity matmuls on the otherwise-idle PE; PSUM->SBUF eviction on the ACT
  engine. DVE only builds selection matrices.
"""
import sys

sys.path.insert(0, "/opt/trn_rl_repo")

from contextlib import ExitStack  # noqa: E402

import numpy as np  # noqa: E402

from concourse import bacc, bass, mybir, tile  # noqa: E402
from concourse.bass_utils import run_bass_kernel_spmd  # noqa: E402

try:
    import ml_dtypes  # noqa: E402

    BF16_NP = ml_dtypes.bfloat16
except ImportError:  # pragma: no cover
    import jax.numpy as jnp  # noqa: E402

    BF16_NP = jnp.bfloat16

P = 128
N_CORES = 8
FP32 = mybir.dt.float32
BF16 = mybir.dt.bfloat16
I16 = mybir.dt.int16

# Problem dims (hardcoded per spec)
B, V, T, FIN = 16, 12288, 4, 64
KV, KT, FOUT = 4, 4, 64
BC = B // N_CORES          # batches per core
F = BC * FOUT              # spmm column width per core (both batches)
C = T * FIN                # z-matmul contraction dim (= 2 chunks of 128)
NT = V // P                # out tiles
NNZ = 9 * V
CH = NNZ // P              # 128-nnz chunks, exact (no padding)
PIECE = 1024               # gather idxs per dma_gather (HW SWDGE ring cap)
DBG_NO_BUILD = False       # reuse one sT for all selection matmuls
DBG_NO_MM = False          # skip selection matmuls
DBG_NO_GATHER = False      # reuse first gather buffer
PSZ_BUFS = 4
PSS_BUFS = 4
SPOOL_BUFS = 8
GPOOL_BUFS = 8
N_PIECES = NNZ // PIECE
SPAN = 1536                # xT columns loaded per DMA in the z phase
ZB = 4 * F                 # zps column count: [z0|z1|z2|z3]


def _preprocess_lap(lap_rows, lap_cols, lap_vals):
    """Sort nnz by row; emit per-(chunk, tile) selection-matmul metadata.

    Chunks are exact 128-nnz groups of the row-sorted nonzeros. A chunk whose
    rows cross an out-tile boundary yields one matmul entry per tile touched;
    lanes outside the entry's tile are masked with rowl = -1.

    Returns (gidx [128, NNZ//16] int16, growl/gval1/gval2 [128, NMM] f32,
    mm_chunk [NMM], mm_tile [NMM]).
    """
    assert lap_rows.shape == (NNZ,)
    order = np.argsort(lap_rows, kind="stable")
    srows = lap_rows[order].astype(np.int64)
    scols = lap_cols[order].astype(np.int64)
    svals = lap_vals[order].astype(np.float32)
    tile_of = srows // P

    mm_chunk, mm_tile = [], []
    growl_cols, gval_cols = [], []
    for k in range(CH):
        s0 = k * P
        tl = tile_of[s0:s0 + P]
        rows = srows[s0:s0 + P]
        vals = svals[s0:s0 + P]
        for t in np.unique(tl):
            mask = tl == t
            rowl = np.where(mask, rows - t * P, -1).astype(np.float32)
            mm_chunk.append(k)
            mm_tile.append(int(t))
            growl_cols.append(rowl)
            gval_cols.append(vals)
    nmm = len(mm_chunk)
    growl = np.stack(growl_cols, axis=1)          # [128, NMM]
    gval = np.stack(gval_cols, axis=1)            # [128, NMM]

    gidx = scols.astype(np.int16).reshape(-1, 16).T.copy()   # [16, NNZ//16]
    gidx = np.tile(gidx, (8, 1))                             # [128, NNZ//16]
    return (np.ascontiguousarray(gidx),
            np.ascontiguousarray(growl),
            np.ascontiguousarray(gval),
            np.ascontiguousarray(2.0 * gval),
            tuple(mm_chunk), tuple(mm_tile))


def build_program(mm_chunk, mm_tile, n_cores=N_CORES, max_phase=3):
    """Build the SPMD Bass program (identical across cores)."""
    nmm = len(mm_chunk)
    # per-tile contiguous mm ranges
    tile_mms = [[] for _ in range(NT)]
    for mi, t in enumerate(mm_tile):
        tile_mms[t].append(mi)

    nc = bacc.Bacc("TRN2", target_bir_lowering=False, debug=False,
                   num_devices=n_cores, dynamic_dma_scratch_size=16384)

    xt_d = nc.dram_tensor("xt", [BC, 2, P, V], BF16, kind="ExternalInput")
    wz_d = nc.dram_tensor("wz", [P, 2 * KV * FOUT], BF16, kind="ExternalInput")
    ident_d = nc.dram_tensor("ident", [P, P], BF16, kind="ExternalInput")
    nident_d = nc.dram_tensor("nident", [P, P], BF16, kind="ExternalInput")
    ones_d = nc.dram_tensor("onescol", [P, P], BF16, kind="ExternalInput")
    biasm_d = nc.dram_tensor("biasmat", [P, F], BF16, kind="ExternalInput")
    iota_d = nc.dram_tensor("iota", [P, P], BF16, kind="ExternalInput")
    gidx_d = nc.dram_tensor("gidx", [P, NNZ // 16], I16, kind="ExternalInput")
    growl_d = nc.dram_tensor("growl", [P, nmm], FP32, kind="ExternalInput")
    gval1_d = nc.dram_tensor("gval1", [P, nmm], FP32, kind="ExternalInput")
    gval2_d = nc.dram_tensor("gval2", [P, nmm], FP32, kind="ExternalInput")
    out_d = nc.dram_tensor("out", [V, F], FP32, kind="ExternalOutput")

    with tile.TileContext(nc) as tc, ExitStack() as ctx:
        dram = ctx.enter_context(tc.tile_pool(name="dram", bufs=1, space="DRAM"))
        b3_d = dram.tile([V, F], BF16, tag="b3d")
        b2_d = dram.tile([V, F], BF16, tag="b2d")
        b1_d = dram.tile([V, F], BF16, tag="b1d")

        const = ctx.enter_context(tc.tile_pool(name="const", bufs=1))
        xsp = ctx.enter_context(tc.tile_pool(name="xsp", bufs=2))
        zres_p = ctx.enter_context(tc.tile_pool(name="zresp", bufs=1))
        gpool = ctx.enter_context(tc.tile_pool(name="gbuf", bufs=GPOOL_BUFS))
        spool = ctx.enter_context(tc.tile_pool(name="sel", bufs=SPOOL_BUFS))
        stg = ctx.enter_context(tc.tile_pool(name="stg", bufs=3))
        ostg = ctx.enter_context(tc.tile_pool(name="ostg", bufs=3))
        psz = ctx.enter_context(tc.tile_pool(name="psz", bufs=PSZ_BUFS, space="PSUM"))
        pss = ctx.enter_context(tc.tile_pool(name="pss", bufs=PSS_BUFS, space="PSUM"))

        def cload(shape, dtype, src, tag):
            t = const.tile(shape, dtype, tag=tag)
            nc.sync.dma_start(t[:], src[:, :])
            return t

        wz_sb = cload([P, 2 * KV * FOUT], BF16, wz_d, "wz")
        ident_sb = cload([P, P], BF16, ident_d, "ident")
        nident_sb = cload([P, P], BF16, nident_d, "nident")
        ones_sb = cload([P, P], BF16, ones_d, "ones")
        biasm_sb = cload([P, F], BF16, biasm_d, "biasm")
        iota_sb = cload([P, P], BF16, iota_d, "iota")
        # big spmm metadata is loaded after the first x spans (see below) so
        # the Z phase's critical path starts immediately
        meta_sb = {}

        def load_meta():
            # split across the SP and ACT HWDGE queues
            t = const.tile([P, NNZ // 16], I16, tag="gidx")
            nc.sync.dma_start(t[:], gidx_d[:, :])
            meta_sb["gidx"] = t
            t2 = const.tile([P, nmm], FP32, tag="growl")
            nc.scalar.dma_start(t2[:], growl_d[:, :])
            meta_sb["growl"] = t2
            t3 = const.tile([P, nmm], FP32, tag="gval1")
            nc.scalar.dma_start(t3[:], gval1_d[:, :])
            meta_sb["gval1"] = t3
            t4 = const.tile([P, nmm], FP32, tag="gval2")
            nc.scalar.dma_start(t4[:], gval2_d[:, :])
            meta_sb["gval2"] = t4

        # z0..z3 resident, bf16: per tile 4 blocks of F columns [z0|z1|z2|z3]
        zres = zres_p.tile([P, NT * ZB], BF16, tag="zres")

        def zslot(vt, j):
            return zres[:, vt * ZB + j * F:vt * ZB + (j + 1) * F]

        zresv = zres[:].rearrange("p (t z) -> p t z", z=ZB)

        def zsrc_group(vt0, n, j):
            """[p, n, F] strided view of z-block j for tiles vt0..vt0+n."""
            return zresv[:, vt0:vt0 + n, j * F:(j + 1) * F]

        def dram_group(d, vt0, n, dt=None):
            """[p, n, F] view of DRAM rows vt0*P..(vt0+n)*P."""
            return d[vt0 * P:(vt0 + n) * P, :].rearrange(
                "(t p) f -> p t f", p=P)

        # ---------- Z phase: z_k = x0 @ w_k, all k, no transposes ----------
        n_spans = V // SPAN
        vt_per_span = SPAN // P
        GRP = 8
        spans = {}

        def load_span(si):
            sp = {}
            for b in range(BC):
                xt = xsp.tile([P, 2, SPAN], BF16, tag=f"x{b}")
                nc.sync.dma_start(
                    xt[:], xt_d[b, :, :, si * SPAN:(si + 1) * SPAN].rearrange(
                        "c p v -> p c v"))
                sp[b] = xt
            spans[si] = sp

        load_span(0)
        for si in range(n_spans):
            if si + 1 < n_spans:
                load_span(si + 1)
            sp = spans.pop(si)
            if si == 1:
                load_meta()
            for vj in range(vt_per_span):
                vt = si * vt_per_span + vj
                off = vj * P
                zps = psz.tile([P, ZB], FP32, tag="zps")
                for bb in range(BC):
                    for k in range(KV):
                        dst = zps[:, k * F + bb * FOUT:k * F + (bb + 1) * FOUT]
                        for cc in range(2):
                            nc.tensor.matmul(
                                dst, lhsT=sp[bb][:, cc, off:off + P],
                                rhs=wz_sb[:, (cc * KV + k) * FOUT:
                                          (cc * KV + k + 1) * FOUT],
                                start=(cc == 0), stop=(cc == 1))
                # evict psum -> zres (DVE lower half, ACT upper half)
                nc.vector.tensor_copy(zres[:, vt * ZB:vt * ZB + 2 * F],
                                      zps[:, 0:2 * F])
                nc.scalar.copy(out=zres[:, vt * ZB + 2 * F:vt * ZB + ZB],
                               in_=zps[:, 2 * F:ZB])
                # b3 = z3 to DRAM (gather source), 8 tiles per DMA.
                # Issued from the ACT queue so it cannot head-of-line block
                # the SP queue's span prefetches.
                if vt % GRP == GRP - 1:
                    nc.scalar.dma_start(dram_group(b3_d, vt - GRP + 1, GRP),
                                        zsrc_group(vt - GRP + 1, GRP, 3))

        # ---------- spmm phases ----------
        def spmm_phase(src_d, vals_sb, extras, sink):
            state = {"gb": None, "base": -1}

            def ensure_piece(k):
                pc = 0 if (DBG_NO_GATHER and state["base"] >= 0) else (k * P) // PIECE
                if state["base"] != pc:
                    gb = gpool.tile([P, PIECE // P, F], BF16, tag="gb")
                    s0 = pc * PIECE
                    nc.gpsimd.dma_gather(
                        out_ap=gb[:],
                        in_ap=src_d[:, :],
                        idxs_ap=gidx_sb[:, s0 // 16:(s0 + PIECE) // 16],
                        num_idxs=PIECE,
                        num_idxs_reg=PIECE,
                        elem_size=F,
                    )
                    state.update(gb=gb, base=pc)
                return state["gb"], pc

            for tt in range(NT):
                mms = tile_mms[tt]
                ps = pss.tile([P, F], FP32, tag="ps")
                for j, mi in enumerate(mms):
                    k = mm_chunk[mi]
                    gb, pc = ensure_piece(k)
                    if not DBG_NO_BUILD:
                        sT = spool.tile([P, P], BF16, tag="sT")
                        nc.vector.tensor_scalar(
                            out=sT[:], in0=iota_sb[:],
                            scalar1=growl_sb[:, mi:mi + 1],
                            scalar2=vals_sb[:, mi:mi + 1],
                            op0=mybir.AluOpType.is_equal,
                            op1=mybir.AluOpType.mult,
                        )
                    else:
                        sT = ident_sb
                    if not DBG_NO_MM:
                        nc.tensor.matmul(ps[:], lhsT=sT[:],
                                         rhs=gb[:, (k - pc * (PIECE // P)) % (PIECE // P), :],
                                         start=(j == 0), stop=False)
                for e, (lhs_sb, rhs_of) in enumerate(extras):
                    nc.tensor.matmul(ps[:], lhsT=lhs_sb[:], rhs=rhs_of(tt),
                                     start=((not mms or DBG_NO_MM) and e == 0),
                                     stop=(e == len(extras) - 1))
                sink(tt, ps)

        # phase 1: b2 = z2 + 2 L b3   (z2 slot becomes b2 in place)
        def sink1(tt, ps):
            nc.scalar.copy(out=zslot(tt, 2), in_=ps[:])
            if tt % GRP == GRP - 1:
                nc.sync.dma_start(dram_group(b2_d, tt - GRP + 1, GRP),
                                  zsrc_group(tt - GRP + 1, GRP, 2))

        gidx_sb = meta_sb["gidx"]
        gval1_sb = meta_sb["gval1"]
        gval2_sb = meta_sb["gval2"]
        growl_sb = meta_sb["growl"]

        if max_phase >= 1:
            spmm_phase(b3_d, gval2_sb,
                       [(ident_sb, lambda tt: zslot(tt, 2))], sink1)

        # phase 2: b1 = z1 + 2 L b2 - b3(=z3)
        st2 = {"t": None}

        def sink2(tt, ps):
            if tt % GRP == 0:
                b1st = stg.tile([P, GRP, F], BF16, tag="b1st")
                st2["t"] = b1st
            nc.scalar.copy(out=st2["t"][:, tt % GRP, :], in_=ps[:])
            if tt % GRP == GRP - 1:
                nc.sync.dma_start(dram_group(b1_d, tt - GRP + 1, GRP),
                                  st2["t"][:])

        if max_phase >= 2:
            spmm_phase(b2_d, gval2_sb,
                       [(ident_sb, lambda tt: zslot(tt, 1)),
                        (nident_sb, lambda tt: zslot(tt, 3))], sink2)

        # phase 3: out = z0 + L b1 - b2(=z2 slot) + bias
        st3 = {"t": None}

        def sink3(tt, ps):
            if tt % GRP == 0:
                ost = ostg.tile([P, GRP, F], FP32, tag="ost")
                st3["t"] = ost
            nc.scalar.copy(out=st3["t"][:, tt % GRP, :], in_=ps[:])
            if tt % GRP == GRP - 1:
                nc.sync.dma_start(dram_group(out_d, tt - GRP + 1, GRP),
                                  st3["t"][:])

        if max_phase >= 3:
            spmm_phase(b1_d, gval1_sb,
                       [(ident_sb, lambda tt: zslot(tt, 0)),
                        (nident_sb, lambda tt: zslot(tt, 2)),
                        (ones_sb, lambda tt: biasm_sb[:])], sink3)

    nc.compile()
    return nc


def make_host_inputs(inputs, weight, bias, lap_vals, lap_rows, lap_cols):
    """Build per-core input maps + metadata. Returns (in_maps, key)."""
    gidx, growl, gval1, gval2, mm_chunk, mm_tile = _preprocess_lap(
        np.asarray(lap_rows), np.asarray(lap_cols),
        np.asarray(lap_vals, np.float32))

    w = np.asarray(weight, np.float32)          # [Fin, Kv, Kt, Fout]
    # wz[c=(t,f)][k, o]; column layout per cc-chunk of c: (cc*KV + k)*FOUT + o
    wzc = np.transpose(w, (2, 0, 1, 3)).reshape(C, KV * FOUT)
    wz = np.ascontiguousarray(
        wzc.reshape(2, P, KV * FOUT).transpose(1, 0, 2).reshape(P, 2 * KV * FOUT)
    ).astype(BF16_NP)

    ident = np.eye(P, dtype=np.float32).astype(BF16_NP)
    nident = (-np.eye(P, dtype=np.float32)).astype(BF16_NP)
    ones_col = np.zeros((P, P), np.float32)
    ones_col[0, :] = 1.0
    ones_col = ones_col.astype(BF16_NP)
    biasm = np.zeros((P, F), np.float32)
    biasm[0, :] = np.tile(np.asarray(bias, np.float32), BC)
    biasm = biasm.astype(BF16_NP)
    iota = np.ascontiguousarray(
        np.broadcast_to(np.arange(P, dtype=np.float32)[None, :], (P, P))
    ).astype(BF16_NP)

    common = {
        "wz": wz, "ident": ident, "nident": nident, "onescol": ones_col,
        "biasmat": biasm, "iota": iota, "gidx": gidx, "growl": growl,
        "gval1": gval1, "gval2": gval2,
    }
    x = np.asarray(inputs, np.float32)          # [B, V, T, Fin]
    in_maps = []
    for r in range(N_CORES):
        xc = x[BC * r:BC * (r + 1)].reshape(BC, V, C)        # [BC, V, 256]
        xt = np.ascontiguousarray(
            xc.transpose(0, 2, 1).reshape(BC, 2, P, V)).astype(BF16_NP)
        m = dict(common)
        m["xt"] = xt
        in_maps.append(m)
    return in_maps, (mm_chunk, mm_tile)


_CACHE = {}


def _get_program(key):
    if key not in _CACHE:
        _CACHE[key] = build_program(key[0], key[1])
    return _CACHE[key]


def kernel(inputs, weight, bias, lap_vals, lap_rows, lap_cols):
    in_maps, key = make_host_inputs(inputs, weight, bias, lap_vals,
                                    lap_rows, lap_cols)
    nc = _get_program(key)
    res = run_bass_kernel_spmd(nc, in_maps, list(range(N_CORES)))
    outs = []
    for r in range(N_CORES):
        o = res.results[r]["out"]                # [V, BC*FOUT]
        outs.append(o.reshape(V, BC, FOUT).transpose(1, 0, 2))
    out = np.concatenate(outs, axis=0)
    return np.ascontiguousarray(out.astype(np.float32))


def time_kernel(inputs_dict, iters=3):
    """Wall-clock repeated executions of the cached program (ns per run)."""
    import time

    in_maps, key = make_host_inputs(**inputs_dict)
    nc = _get_program(key)
    times = []
    for _ in range(iters):
        t0 = time.perf_counter()
        run_bass_kernel_spmd(nc, in_maps, list(range(N_CORES)))
        times.append(time.perf_counter() - t0)
    return min(times) * 1e9


# revision 22
# speedup vs baseline: 1.5222x; 1.0037x over previous
"""Trainium2 Bass kernel for ConvChebTemp (Chebyshev graph conv, temporal weights).

Math: out[b,v,o] = sum_{k,t,f} T_k(L)x0[:,t,f,b] w[f,k,t,o] + bias[o]
with x0 = inputs permuted to [V, T*Fin*B] and T_k the Chebyshev recurrence.

Clenshaw reformulation (weights contracted first):
  z_k[v, b, o] = sum_{t,f} x0[v,t,f,b] w[f,k,t,o]
  b3 = z3; b2 = z2 + 2 L b3; b1 = z1 + 2 L b2 - b3; out = z0 + L b1 - b2 + bias

Design (one NeuronCore per 2 batches, 8 cores data-parallel over B=16):
- Host pre-transposes x to [c, v] layout in bf16, so the z matmuls need no
  on-chip transposes: zps[v, (k,b,o)] accumulates directly in PSUM with a
  column layout [z0|z1|z2|z3] (each 128 = [b0 64|b1 64]).
- z0..z3 kept SBUF-resident in bf16 (zres); b3/b2/b1 gather sources in DRAM
  bf16 (256 B rows).
- SpMM via dma_gather (sorted-by-row, exact-NNZ chunks that may span
  out-tile boundaries; spanning chunks get one selection matmul per tile,
  with out-of-tile lanes masked by rowl=-1) + bf16 selection matmuls.
- Combine terms (z adds/subs, bias) are folded into the PSUM accumulation as
  identity matmuls on the otherwise-idle PE; PSUM->SBUF eviction on the ACT
  engine. DVE only builds selection matrices.
"""
import sys

sys.path.insert(0, "/opt/trn_rl_repo")

from contextlib import ExitStack  # noqa: E402

import numpy as np  # noqa: E402

from concourse import bacc, bass, mybir, tile  # noqa: E402
from concourse.bass_utils import run_bass_kernel_spmd  # noqa: E402

try:
    import ml_dtypes  # noqa: E402

    BF16_NP = ml_dtypes.bfloat16
except ImportError:  # pragma: no cover
    import jax.numpy as jnp  # noqa: E402

    BF16_NP = jnp.bfloat16

P = 128
N_CORES = 8
FP32 = mybir.dt.float32
BF16 = mybir.dt.bfloat16
I16 = mybir.dt.int16

# Problem dims (hardcoded per spec)
B, V, T, FIN = 16, 12288, 4, 64
KV, KT, FOUT = 4, 4, 64
BC = B // N_CORES          # batches per core
F = BC * FOUT              # spmm column width per core (both batches)
C = T * FIN                # z-matmul contraction dim (= 2 chunks of 128)
NT = V // P                # out tiles
NNZ = 9 * V
CH = NNZ // P              # 128-nnz chunks, exact (no padding)
PIECE = 1024               # gather idxs per dma_gather (HW SWDGE ring cap)
DBG_NO_BUILD = False       # reuse one sT for all selection matmuls
DBG_NO_MM = False          # skip selection matmuls
DBG_NO_GATHER = False      # reuse first gather buffer
PSZ_BUFS = 4
PSS_BUFS = 4
SPOOL_BUFS = 8
GPOOL_BUFS = 8
N_PIECES = NNZ // PIECE
SPAN = 1536                # xT columns loaded per DMA in the z phase
ZB = 4 * F                 # zps column count: [z0|z1|z2|z3]


def _preprocess_lap(lap_rows, lap_cols, lap_vals):
    """Sort nnz by row; emit per-(chunk, tile) selection-matmul metadata.

    Chunks are exact 128-nnz groups of the row-sorted nonzeros. A chunk whose
    rows cross an out-tile boundary yields one matmul entry per tile touched;
    lanes outside the entry's tile are masked with rowl = -1.

    Returns (gidx [128, NNZ//16] int16, growl/gval1/gval2 [128, NMM] f32,
    mm_chunk [NMM], mm_tile [NMM]).
    """
    assert lap_rows.shape == (NNZ,)
    order = np.argsort(lap_rows, kind="stable")
    srows = lap_rows[order].astype(np.int64)
    scols = lap_cols[order].astype(np.int64)
    svals = lap_vals[order].astype(np.float32)
    tile_of = srows // P

    mm_chunk, mm_tile = [], []
    growl_cols, gval_cols = [], []
    for k in range(CH):
        s0 = k * P
        tl = tile_of[s0:s0 + P]
        rows = srows[s0:s0 + P]
        vals = svals[s0:s0 + P]
        for t in np.unique(tl):
            mask = tl == t
            rowl = np.where(mask, rows - t * P, -1).astype(np.float32)
            mm_chunk.append(k)
            mm_tile.append(int(t))
            growl_cols.append(rowl)
            gval_cols.append(vals)
    nmm = len(mm_chunk)
    growl = np.stack(growl_cols, axis=1)          # [128, NMM]
    gval = np.stack(gval_cols, axis=1)            # [128, NMM]

    gidx = scols.astype(np.int16).reshape(-1, 16).T.copy()   # [16, NNZ//16]
    gidx = np.tile(gidx, (8, 1))                             # [128, NNZ//16]
    return (np.ascontiguousarray(gidx),
            np.ascontiguousarray(growl),
            np.ascontiguousarray(gval),
            np.ascontiguousarray(2.0 * gval),
            tuple(mm_chunk), tuple(mm_tile))


def build_program(mm_chunk, mm_tile, n_cores=N_CORES, max_phase=3):
    """Build the SPMD Bass program (identical across cores)."""
    nmm = len(mm_chunk)
    # per-tile contiguous mm ranges
    tile_mms = [[] for _ in range(NT)]
    for mi, t in enumerate(mm_tile):
        tile_mms[t].append(mi)

    nc = bacc.Bacc("TRN2", target_bir_lowering=False, debug=False,
                   num_devices=n_cores, dynamic_dma_scratch_size=16384)

    xt_d = nc.dram_tensor("xt", [BC, 2, P, V], BF16, kind="ExternalInput")
    wz_d = nc.dram_tensor("wz", [P, 2 * KV * FOUT], BF16, kind="ExternalInput")
    ident_d = nc.dram_tensor("ident", [P, P], BF16, kind="ExternalInput")
    nident_d = nc.dram_tensor("nident", [P, P], BF16, kind="ExternalInput")
    ones_d = nc.dram_tensor("onescol", [P, P], BF16, kind="ExternalInput")
    biasm_d = nc.dram_tensor("biasmat", [P, F], BF16, kind="ExternalInput")
    iota_d = nc.dram_tensor("iota", [P, P], BF16, kind="ExternalInput")
    gidx_d = nc.dram_tensor("gidx", [P, NNZ // 16], I16, kind="ExternalInput")
    growl_d = nc.dram_tensor("growl", [P, nmm], FP32, kind="ExternalInput")
    gval1_d = nc.dram_tensor("gval1", [P, nmm], FP32, kind="ExternalInput")
    gval2_d = nc.dram_tensor("gval2", [P, nmm], FP32, kind="ExternalInput")
    out_d = nc.dram_tensor("out", [V, F], FP32, kind="ExternalOutput")

    with tile.TileContext(nc) as tc, ExitStack() as ctx:
        dram = ctx.enter_context(tc.tile_pool(name="dram", bufs=1, space="DRAM"))
        b3_d = dram.tile([V, F], BF16, tag="b3d")
        b2_d = dram.tile([V, F], BF16, tag="b2d")
        b1_d = dram.tile([V, F], BF16, tag="b1d")

        const = ctx.enter_context(tc.tile_pool(name="const", bufs=1))
        xsp = ctx.enter_context(tc.tile_pool(name="xsp", bufs=3))
        zres_p = ctx.enter_context(tc.tile_pool(name="zresp", bufs=1))
        gpool = ctx.enter_context(tc.tile_pool(name="gbuf", bufs=GPOOL_BUFS))
        spool = ctx.enter_context(tc.tile_pool(name="sel", bufs=SPOOL_BUFS))
        stg = ctx.enter_context(tc.tile_pool(name="stg", bufs=3))
        ostg = ctx.enter_context(tc.tile_pool(name="ostg", bufs=3))
        psz = ctx.enter_context(tc.tile_pool(name="psz", bufs=PSZ_BUFS, space="PSUM"))
        pss = ctx.enter_context(tc.tile_pool(name="pss", bufs=PSS_BUFS, space="PSUM"))

        def cload(shape, dtype, src, tag):
            t = const.tile(shape, dtype, tag=tag)
            nc.sync.dma_start(t[:], src[:, :])
            return t

        wz_sb = cload([P, 2 * KV * FOUT], BF16, wz_d, "wz")
        ident_sb = cload([P, P], BF16, ident_d, "ident")
        nident_sb = cload([P, P], BF16, nident_d, "nident")
        ones_sb = cload([P, P], BF16, ones_d, "ones")
        biasm_sb = cload([P, F], BF16, biasm_d, "biasm")
        iota_sb = cload([P, P], BF16, iota_d, "iota")
        # big spmm metadata is loaded after the first x spans (see below) so
        # the Z phase's critical path starts immediately
        meta_sb = {}

        def load_meta():
            # split across the SP and ACT HWDGE queues
            t = const.tile([P, NNZ // 16], I16, tag="gidx")
            nc.sync.dma_start(t[:], gidx_d[:, :])
            meta_sb["gidx"] = t
            t2 = const.tile([P, nmm], FP32, tag="growl")
            nc.scalar.dma_start(t2[:], growl_d[:, :])
            meta_sb["growl"] = t2
            t3 = const.tile([P, nmm], FP32, tag="gval1")
            nc.scalar.dma_start(t3[:], gval1_d[:, :])
            meta_sb["gval1"] = t3
            t4 = const.tile([P, nmm], FP32, tag="gval2")
            nc.scalar.dma_start(t4[:], gval2_d[:, :])
            meta_sb["gval2"] = t4

        # z0..z3 resident, bf16: per tile 4 blocks of F columns [z0|z1|z2|z3]
        zres = zres_p.tile([P, NT * ZB], BF16, tag="zres")

        def zslot(vt, j):
            return zres[:, vt * ZB + j * F:vt * ZB + (j + 1) * F]

        zresv = zres[:].rearrange("p (t z) -> p t z", z=ZB)

        def zsrc_group(vt0, n, j):
            """[p, n, F] strided view of z-block j for tiles vt0..vt0+n."""
            return zresv[:, vt0:vt0 + n, j * F:(j + 1) * F]

        def dram_group(d, vt0, n, dt=None):
            """[p, n, F] view of DRAM rows vt0*P..(vt0+n)*P."""
            return d[vt0 * P:(vt0 + n) * P, :].rearrange(
                "(t p) f -> p t f", p=P)

        # ---------- Z phase: z_k = x0 @ w_k, all k, no transposes ----------
        n_spans = V // SPAN
        vt_per_span = SPAN // P
        GRP = 8
        spans = {}

        def load_span(si):
            sp = {}
            for b in range(BC):
                xt = xsp.tile([P, 2, SPAN], BF16, tag=f"x{b}")
                nc.sync.dma_start(
                    xt[:], xt_d[b, :, :, si * SPAN:(si + 1) * SPAN].rearrange(
                        "c p v -> p c v"))
                sp[b] = xt
            spans[si] = sp

        load_span(0)
        for si in range(n_spans):
            if si + 1 < n_spans:
                load_span(si + 1)
            sp = spans.pop(si)
            if si == 1:
                load_meta()
            for vj in range(vt_per_span):
                vt = si * vt_per_span + vj
                off = vj * P
                zps = psz.tile([P, ZB], FP32, tag="zps")
                for bb in range(BC):
                    for k in range(KV):
                        dst = zps[:, k * F + bb * FOUT:k * F + (bb + 1) * FOUT]
                        for cc in range(2):
                            nc.tensor.matmul(
                                dst, lhsT=sp[bb][:, cc, off:off + P],
                                rhs=wz_sb[:, (cc * KV + k) * FOUT:
                                          (cc * KV + k + 1) * FOUT],
                                start=(cc == 0), stop=(cc == 1))
                # evict psum -> zres (DVE lower half, ACT upper half)
                nc.vector.tensor_copy(zres[:, vt * ZB:vt * ZB + 2 * F],
                                      zps[:, 0:2 * F])
                nc.scalar.copy(out=zres[:, vt * ZB + 2 * F:vt * ZB + ZB],
                               in_=zps[:, 2 * F:ZB])
                # b3 = z3 to DRAM (gather source), 8 tiles per DMA.
                # Issued from the ACT queue so it cannot head-of-line block
                # the SP queue's span prefetches.
                if vt % GRP == GRP - 1:
                    nc.scalar.dma_start(dram_group(b3_d, vt - GRP + 1, GRP),
                                        zsrc_group(vt - GRP + 1, GRP, 3))

        # ---------- spmm phases ----------
        def spmm_phase(src_d, vals_sb, extras, sink):
            state = {"gb": None, "base": -1}

            def ensure_piece(k):
                pc = 0 if (DBG_NO_GATHER and state["base"] >= 0) else (k * P) // PIECE
                if state["base"] != pc:
                    gb = gpool.tile([P, PIECE // P, F], BF16, tag="gb")
                    s0 = pc * PIECE
                    nc.gpsimd.dma_gather(
                        out_ap=gb[:],
                        in_ap=src_d[:, :],
                        idxs_ap=gidx_sb[:, s0 // 16:(s0 + PIECE) // 16],
                        num_idxs=PIECE,
                        num_idxs_reg=PIECE,
                        elem_size=F,
                    )
                    state.update(gb=gb, base=pc)
                return state["gb"], pc

            for tt in range(NT):
                mms = tile_mms[tt]
                ps = pss.tile([P, F], FP32, tag="ps")
                for j, mi in enumerate(mms):
                    k = mm_chunk[mi]
                    gb, pc = ensure_piece(k)
                    if not DBG_NO_BUILD:
                        sT = spool.tile([P, P], BF16, tag="sT")
                        nc.vector.tensor_scalar(
                            out=sT[:], in0=iota_sb[:],
                            scalar1=growl_sb[:, mi:mi + 1],
                            scalar2=vals_sb[:, mi:mi + 1],
                            op0=mybir.AluOpType.is_equal,
                            op1=mybir.AluOpType.mult,
                        )
                    else:
                        sT = ident_sb
                    if not DBG_NO_MM:
                        nc.tensor.matmul(ps[:], lhsT=sT[:],
                                         rhs=gb[:, (k - pc * (PIECE // P)) % (PIECE // P), :],
                                         start=(j == 0), stop=False)
                for e, (lhs_sb, rhs_of) in enumerate(extras):
                    nc.tensor.matmul(ps[:], lhsT=lhs_sb[:], rhs=rhs_of(tt),
                                     start=((not mms or DBG_NO_MM) and e == 0),
                                     stop=(e == len(extras) - 1))
                sink(tt, ps)

        # phase 1: b2 = z2 + 2 L b3   (z2 slot becomes b2 in place)
        def sink1(tt, ps):
            nc.scalar.copy(out=zslot(tt, 2), in_=ps[:])
            if tt % GRP == GRP - 1:
                nc.sync.dma_start(dram_group(b2_d, tt - GRP + 1, GRP),
                                  zsrc_group(tt - GRP + 1, GRP, 2))

        gidx_sb = meta_sb["gidx"]
        gval1_sb = meta_sb["gval1"]
        gval2_sb = meta_sb["gval2"]
        growl_sb = meta_sb["growl"]

        if max_phase >= 1:
            spmm_phase(b3_d, gval2_sb,
                       [(ident_sb, lambda tt: zslot(tt, 2))], sink1)

        # phase 2: b1 = z1 + 2 L b2 - b3(=z3)
        st2 = {"t": None}

        def sink2(tt, ps):
            if tt % GRP == 0:
                b1st = stg.tile([P, GRP, F], BF16, tag="b1st")
                st2["t"] = b1st
            nc.scalar.copy(out=st2["t"][:, tt % GRP, :], in_=ps[:])
            if tt % GRP == GRP - 1:
                nc.sync.dma_start(dram_group(b1_d, tt - GRP + 1, GRP),
                                  st2["t"][:])

        if max_phase >= 2:
            spmm_phase(b2_d, gval2_sb,
                       [(ident_sb, lambda tt: zslot(tt, 1)),
                        (nident_sb, lambda tt: zslot(tt, 3))], sink2)

        # phase 3: out = z0 + L b1 - b2(=z2 slot) + bias
        st3 = {"t": None}

        def sink3(tt, ps):
            if tt % GRP == 0:
                ost = ostg.tile([P, GRP, F], FP32, tag="ost")
                st3["t"] = ost
            nc.scalar.copy(out=st3["t"][:, tt % GRP, :], in_=ps[:])
            if tt % GRP == GRP - 1:
                nc.sync.dma_start(dram_group(out_d, tt - GRP + 1, GRP),
                                  st3["t"][:])

        if max_phase >= 3:
            spmm_phase(b1_d, gval1_sb,
                       [(ident_sb, lambda tt: zslot(tt, 0)),
                        (nident_sb, lambda tt: zslot(tt, 2)),
                        (ones_sb, lambda tt: biasm_sb[:])], sink3)

    nc.compile()
    return nc


def make_host_inputs(inputs, weight, bias, lap_vals, lap_rows, lap_cols):
    """Build per-core input maps + metadata. Returns (in_maps, key)."""
    gidx, growl, gval1, gval2, mm_chunk, mm_tile = _preprocess_lap(
        np.asarray(lap_rows), np.asarray(lap_cols),
        np.asarray(lap_vals, np.float32))

    w = np.asarray(weight, np.float32)          # [Fin, Kv, Kt, Fout]
    # wz[c=(t,f)][k, o]; column layout per cc-chunk of c: (cc*KV + k)*FOUT + o
    wzc = np.transpose(w, (2, 0, 1, 3)).reshape(C, KV * FOUT)
    wz = np.ascontiguousarray(
        wzc.reshape(2, P, KV * FOUT).transpose(1, 0, 2).reshape(P, 2 * KV * FOUT)
    ).astype(BF16_NP)

    ident = np.eye(P, dtype=np.float32).astype(BF16_NP)
    nident = (-np.eye(P, dtype=np.float32)).astype(BF16_NP)
    ones_col = np.zeros((P, P), np.float32)
    ones_col[0, :] = 1.0
    ones_col = ones_col.astype(BF16_NP)
    biasm = np.zeros((P, F), np.float32)
    biasm[0, :] = np.tile(np.asarray(bias, np.float32), BC)
    biasm = biasm.astype(BF16_NP)
    iota = np.ascontiguousarray(
        np.broadcast_to(np.arange(P, dtype=np.float32)[None, :], (P, P))
    ).astype(BF16_NP)

    common = {
        "wz": wz, "ident": ident, "nident": nident, "onescol": ones_col,
        "biasmat": biasm, "iota": iota, "gidx": gidx, "growl": growl,
        "gval1": gval1, "gval2": gval2,
    }
    x = np.asarray(inputs, np.float32)          # [B, V, T, Fin]
    in_maps = []
    for r in range(N_CORES):
        xc = x[BC * r:BC * (r + 1)].reshape(BC, V, C)        # [BC, V, 256]
        xt = np.ascontiguousarray(
            xc.transpose(0, 2, 1).reshape(BC, 2, P, V)).astype(BF16_NP)
        m = dict(common)
        m["xt"] = xt
        in_maps.append(m)
    return in_maps, (mm_chunk, mm_tile)


_CACHE = {}


def _get_program(key):
    if key not in _CACHE:
        _CACHE[key] = build_program(key[0], key[1])
    return _CACHE[key]


def kernel(inputs, weight, bias, lap_vals, lap_rows, lap_cols):
    in_maps, key = make_host_inputs(inputs, weight, bias, lap_vals,
                                    lap_rows, lap_cols)
    nc = _get_program(key)
    res = run_bass_kernel_spmd(nc, in_maps, list(range(N_CORES)))
    outs = []
    for r in range(N_CORES):
        o = res.results[r]["out"]                # [V, BC*FOUT]
        outs.append(o.reshape(V, BC, FOUT).transpose(1, 0, 2))
    out = np.concatenate(outs, axis=0)
    return np.ascontiguousarray(out.astype(np.float32))


def time_kernel(inputs_dict, iters=3):
    """Wall-clock repeated executions of the cached program (ns per run)."""
    import time

    in_maps, key = make_host_inputs(**inputs_dict)
    nc = _get_program(key)
    times = []
    for _ in range(iters):
        t0 = time.perf_counter()
        run_bass_kernel_spmd(nc, in_maps, list(range(N_CORES)))
        times.append(time.perf_counter() - t0)
    return min(times) * 1e9


# revision 24
# speedup vs baseline: 1.5237x; 1.0010x over previous
"""Trainium2 Bass kernel for ConvChebTemp (Chebyshev graph conv, temporal weights).

Math: out[b,v,o] = sum_{k,t,f} T_k(L)x0[:,t,f,b] w[f,k,t,o] + bias[o]
with x0 = inputs permuted to [V, T*Fin*B] and T_k the Chebyshev recurrence.

Clenshaw reformulation (weights contracted first):
  z_k[v, b, o] = sum_{t,f} x0[v,t,f,b] w[f,k,t,o]
  b3 = z3; b2 = z2 + 2 L b3; b1 = z1 + 2 L b2 - b3; out = z0 + L b1 - b2 + bias

Design (one NeuronCore per 2 batches, 8 cores data-parallel over B=16):
- Host pre-transposes x to [c, v] layout in bf16, so the z matmuls need no
  on-chip transposes: zps[v, (k,b,o)] accumulates directly in PSUM with a
  column layout [z0|z1|z2|z3] (each 128 = [b0 64|b1 64]).
- z0..z3 kept SBUF-resident in bf16 (zres); b3/b2/b1 gather sources in DRAM
  bf16 (256 B rows).
- SpMM via dma_gather (sorted-by-row, exact-NNZ chunks that may span
  out-tile boundaries; spanning chunks get one selection matmul per tile,
  with out-of-tile lanes masked by rowl=-1) + bf16 selection matmuls.
- Combine terms (z adds/subs, bias) are folded into the PSUM accumulation as
  identity matmuls on the otherwise-idle PE; PSUM->SBUF eviction on the ACT
  engine. DVE only builds selection matrices.
"""
import sys

sys.path.insert(0, "/opt/trn_rl_repo")

from contextlib import ExitStack  # noqa: E402

import numpy as np  # noqa: E402

from concourse import bacc, bass, mybir, tile  # noqa: E402
from concourse.bass_utils import run_bass_kernel_spmd  # noqa: E402

try:
    import ml_dtypes  # noqa: E402

    BF16_NP = ml_dtypes.bfloat16
except ImportError:  # pragma: no cover
    import jax.numpy as jnp  # noqa: E402

    BF16_NP = jnp.bfloat16

P = 128
N_CORES = 8
FP32 = mybir.dt.float32
BF16 = mybir.dt.bfloat16
I16 = mybir.dt.int16

# Problem dims (hardcoded per spec)
B, V, T, FIN = 16, 12288, 4, 64
KV, KT, FOUT = 4, 4, 64
BC = B // N_CORES          # batches per core
F = BC * FOUT              # spmm column width per core (both batches)
C = T * FIN                # z-matmul contraction dim (= 2 chunks of 128)
NT = V // P                # out tiles
NNZ = 9 * V
CH = NNZ // P              # 128-nnz chunks, exact (no padding)
PIECE = 1024               # gather idxs per dma_gather (HW SWDGE ring cap)
DBG_NO_BUILD = False       # reuse one sT for all selection matmuls
DBG_NO_MM = False          # skip selection matmuls
DBG_NO_GATHER = False      # reuse first gather buffer
PSZ_BUFS = 4
PSS_BUFS = 4
SPOOL_BUFS = 8
GPOOL_BUFS = 8
N_PIECES = NNZ // PIECE
SPAN = 1536                # xT columns loaded per DMA in the z phase
ZB = 4 * F                 # zps column count: [z0|z1|z2|z3]


def _preprocess_lap(lap_rows, lap_cols, lap_vals):
    """Sort nnz by row; emit per-(chunk, tile) selection-matmul metadata.

    Chunks are exact 128-nnz groups of the row-sorted nonzeros. A chunk whose
    rows cross an out-tile boundary yields one matmul entry per tile touched;
    lanes outside the entry's tile are masked with rowl = -1.

    Returns (gidx [128, NNZ//16] int16, growl/gval1/gval2 [128, NMM] f32,
    mm_chunk [NMM], mm_tile [NMM]).
    """
    assert lap_rows.shape == (NNZ,)
    order = np.argsort(lap_rows, kind="stable")
    srows = lap_rows[order].astype(np.int64)
    scols = lap_cols[order].astype(np.int64)
    svals = lap_vals[order].astype(np.float32)
    tile_of = srows // P

    mm_chunk, mm_tile = [], []
    growl_cols, gval_cols = [], []
    for k in range(CH):
        s0 = k * P
        tl = tile_of[s0:s0 + P]
        rows = srows[s0:s0 + P]
        vals = svals[s0:s0 + P]
        for t in np.unique(tl):
            mask = tl == t
            rowl = np.where(mask, rows - t * P, -1).astype(np.float32)
            mm_chunk.append(k)
            mm_tile.append(int(t))
            growl_cols.append(rowl)
            gval_cols.append(vals)
    nmm = len(mm_chunk)
    growl = np.stack(growl_cols, axis=1)          # [128, NMM]
    gval = np.stack(gval_cols, axis=1)            # [128, NMM]

    gidx = scols.astype(np.int16).reshape(-1, 16).T.copy()   # [16, NNZ//16]
    gidx = np.tile(gidx, (8, 1))                             # [128, NNZ//16]
    return (np.ascontiguousarray(gidx),
            np.ascontiguousarray(growl),
            np.ascontiguousarray(gval),
            np.ascontiguousarray(2.0 * gval),
            tuple(mm_chunk), tuple(mm_tile))


def build_program(mm_chunk, mm_tile, n_cores=N_CORES, max_phase=3):
    """Build the SPMD Bass program (identical across cores)."""
    nmm = len(mm_chunk)
    # per-tile contiguous mm ranges
    tile_mms = [[] for _ in range(NT)]
    for mi, t in enumerate(mm_tile):
        tile_mms[t].append(mi)

    nc = bacc.Bacc("TRN2", target_bir_lowering=False, debug=False,
                   num_devices=n_cores, dynamic_dma_scratch_size=16384)

    xt_d = nc.dram_tensor("xt", [BC, 2, P, V], BF16, kind="ExternalInput")
    wz_d = nc.dram_tensor("wz", [P, 2 * KV * FOUT], BF16, kind="ExternalInput")
    ident_d = nc.dram_tensor("ident", [P, P], BF16, kind="ExternalInput")
    nident_d = nc.dram_tensor("nident", [P, P], BF16, kind="ExternalInput")
    ones_d = nc.dram_tensor("onescol", [P, P], BF16, kind="ExternalInput")
    biasm_d = nc.dram_tensor("biasmat", [P, F], BF16, kind="ExternalInput")
    iota_d = nc.dram_tensor("iota", [P, P], BF16, kind="ExternalInput")
    gidx_d = nc.dram_tensor("gidx", [P, NNZ // 16], I16, kind="ExternalInput")
    growl_d = nc.dram_tensor("growl", [P, nmm], FP32, kind="ExternalInput")
    gval1_d = nc.dram_tensor("gval1", [P, nmm], FP32, kind="ExternalInput")
    gval2_d = nc.dram_tensor("gval2", [P, nmm], FP32, kind="ExternalInput")
    out_d = nc.dram_tensor("out", [V, F], FP32, kind="ExternalOutput")

    with tile.TileContext(nc) as tc, ExitStack() as ctx:
        dram = ctx.enter_context(tc.tile_pool(name="dram", bufs=1, space="DRAM"))
        b3_d = dram.tile([V, F], BF16, tag="b3d")
        b2_d = dram.tile([V, F], BF16, tag="b2d")
        b1_d = dram.tile([V, F], BF16, tag="b1d")

        const = ctx.enter_context(tc.tile_pool(name="const", bufs=1))
        xsp = ctx.enter_context(tc.tile_pool(name="xsp", bufs=3))
        zres_p = ctx.enter_context(tc.tile_pool(name="zresp", bufs=1))
        gpool = ctx.enter_context(tc.tile_pool(name="gbuf", bufs=GPOOL_BUFS))
        spool = ctx.enter_context(tc.tile_pool(name="sel", bufs=SPOOL_BUFS))
        stg = ctx.enter_context(tc.tile_pool(name="stg", bufs=3))
        ostg = ctx.enter_context(tc.tile_pool(name="ostg", bufs=3))
        psz = ctx.enter_context(tc.tile_pool(name="psz", bufs=PSZ_BUFS, space="PSUM"))
        pss = ctx.enter_context(tc.tile_pool(name="pss", bufs=PSS_BUFS, space="PSUM"))

        def cload(shape, dtype, src, tag):
            t = const.tile(shape, dtype, tag=tag)
            nc.sync.dma_start(t[:], src[:, :])
            return t

        wz_sb = cload([P, 2 * KV * FOUT], BF16, wz_d, "wz")
        ident_sb = cload([P, P], BF16, ident_d, "ident")
        nident_sb = cload([P, P], BF16, nident_d, "nident")
        ones_sb = cload([P, P], BF16, ones_d, "ones")
        biasm_sb = cload([P, F], BF16, biasm_d, "biasm")
        iota_sb = cload([P, P], BF16, iota_d, "iota")
        # big spmm metadata is loaded after the first x spans (see below) so
        # the Z phase's critical path starts immediately
        meta_sb = {}

        def load_meta():
            # split across the SP and ACT HWDGE queues
            t = const.tile([P, NNZ // 16], I16, tag="gidx")
            nc.sync.dma_start(t[:], gidx_d[:, :])
            meta_sb["gidx"] = t
            t2 = const.tile([P, nmm], FP32, tag="growl")
            nc.scalar.dma_start(t2[:], growl_d[:, :])
            meta_sb["growl"] = t2
            t3 = const.tile([P, nmm], FP32, tag="gval1")
            nc.scalar.dma_start(t3[:], gval1_d[:, :])
            meta_sb["gval1"] = t3
            t4 = const.tile([P, nmm], FP32, tag="gval2")
            nc.scalar.dma_start(t4[:], gval2_d[:, :])
            meta_sb["gval2"] = t4

        # z0..z3 resident, bf16: per tile 4 blocks of F columns [z0|z1|z2|z3]
        zres = zres_p.tile([P, NT * ZB], BF16, tag="zres")

        def zslot(vt, j):
            return zres[:, vt * ZB + j * F:vt * ZB + (j + 1) * F]

        zresv = zres[:].rearrange("p (t z) -> p t z", z=ZB)

        def zsrc_group(vt0, n, j):
            """[p, n, F] strided view of z-block j for tiles vt0..vt0+n."""
            return zresv[:, vt0:vt0 + n, j * F:(j + 1) * F]

        def dram_group(d, vt0, n, dt=None):
            """[p, n, F] view of DRAM rows vt0*P..(vt0+n)*P."""
            return d[vt0 * P:(vt0 + n) * P, :].rearrange(
                "(t p) f -> p t f", p=P)

        # ---------- Z phase: z_k = x0 @ w_k, all k, no transposes ----------
        n_spans = V // SPAN
        vt_per_span = SPAN // P
        GRP = 8
        spans = {}

        def load_span(si):
            sp = {}
            for b in range(BC):
                xt = xsp.tile([P, 2, SPAN], BF16, tag=f"x{b}")
                nc.sync.dma_start(
                    xt[:], xt_d[b, :, :, si * SPAN:(si + 1) * SPAN].rearrange(
                        "c p v -> p c v"))
                sp[b] = xt
            spans[si] = sp

        load_span(0)
        for si in range(n_spans):
            if si + 1 < n_spans:
                load_span(si + 1)
            sp = spans.pop(si)
            if si == 1:
                load_meta()
            for vj in range(vt_per_span):
                vt = si * vt_per_span + vj
                off = vj * P
                zps = psz.tile([P, ZB], FP32, tag="zps")
                for bb in range(BC):
                    for k in range(KV):
                        dst = zps[:, k * F + bb * FOUT:k * F + (bb + 1) * FOUT]
                        for cc in range(2):
                            nc.tensor.matmul(
                                dst, lhsT=sp[bb][:, cc, off:off + P],
                                rhs=wz_sb[:, (cc * KV + k) * FOUT:
                                          (cc * KV + k + 1) * FOUT],
                                start=(cc == 0), stop=(cc == 1))
                # evict psum -> zres (DVE lower half, ACT upper half)
                nc.vector.tensor_copy(zres[:, vt * ZB:vt * ZB + 2 * F],
                                      zps[:, 0:2 * F])
                nc.scalar.copy(out=zres[:, vt * ZB + 2 * F:vt * ZB + ZB],
                               in_=zps[:, 2 * F:ZB])
                # b3 = z3 to DRAM (gather source), 8 tiles per DMA.
                # Issued from the ACT queue so it cannot head-of-line block
                # the SP queue's span prefetches.
                if vt % GRP == GRP - 1:
                    nc.scalar.dma_start(dram_group(b3_d, vt - GRP + 1, GRP),
                                        zsrc_group(vt - GRP + 1, GRP, 3))

        # ---------- spmm phases ----------
        # piece schedule: small pieces at the phase edges shorten the serial
        # lead-in (first data sooner) and tail (fewer consumers after the
        # last transfer); 1024 is the HW SWDGE ring cap.
        piece_lens = [1024] * 107 + [512, 512]
        assert sum(piece_lens) == NNZ
        piece_start = np.concatenate([[0], np.cumsum(piece_lens)])
        chunk_piece = np.searchsorted(piece_start, np.arange(CH) * P, "right") - 1
        chunk_local = (np.arange(CH) * P - piece_start[chunk_piece]) // P

        def spmm_phase(src_d, vals_sb, extras, sink):
            state = {"gb": None, "base": -1}

            def ensure_piece(k):
                pc = 0 if (DBG_NO_GATHER and state["base"] >= 0) else int(chunk_piece[k])
                if state["base"] != pc:
                    plen = piece_lens[pc]
                    gb = gpool.tile([P, plen // P, F], BF16, tag="gb")
                    s0 = int(piece_start[pc])
                    nc.gpsimd.dma_gather(
                        out_ap=gb[:],
                        in_ap=src_d[:, :],
                        idxs_ap=gidx_sb[:, s0 // 16:(s0 + plen) // 16],
                        num_idxs=plen,
                        num_idxs_reg=plen,
                        elem_size=F,
                    )
                    state.update(gb=gb, base=pc)
                return state["gb"], pc

            for tt in range(NT):
                mms = tile_mms[tt]
                ps = pss.tile([P, F], FP32, tag="ps")
                for j, mi in enumerate(mms):
                    k = mm_chunk[mi]
                    gb, pc = ensure_piece(k)
                    if not DBG_NO_BUILD:
                        sT = spool.tile([P, P], BF16, tag="sT")
                        nc.vector.tensor_scalar(
                            out=sT[:], in0=iota_sb[:],
                            scalar1=growl_sb[:, mi:mi + 1],
                            scalar2=vals_sb[:, mi:mi + 1],
                            op0=mybir.AluOpType.is_equal,
                            op1=mybir.AluOpType.mult,
                        )
                    else:
                        sT = ident_sb
                    if not DBG_NO_MM:
                        nc.tensor.matmul(ps[:], lhsT=sT[:],
                                         rhs=gb[:, int(chunk_local[k]), :],
                                         start=(j == 0), stop=False)
                for e, (lhs_sb, rhs_of) in enumerate(extras):
                    nc.tensor.matmul(ps[:], lhsT=lhs_sb[:], rhs=rhs_of(tt),
                                     start=((not mms or DBG_NO_MM) and e == 0),
                                     stop=(e == len(extras) - 1))
                sink(tt, ps)

        # phase 1: b2 = z2 + 2 L b3   (z2 slot becomes b2 in place)
        def sink1(tt, ps):
            nc.scalar.copy(out=zslot(tt, 2), in_=ps[:])
            if tt % GRP == GRP - 1:
                nc.sync.dma_start(dram_group(b2_d, tt - GRP + 1, GRP),
                                  zsrc_group(tt - GRP + 1, GRP, 2))

        gidx_sb = meta_sb["gidx"]
        gval1_sb = meta_sb["gval1"]
        gval2_sb = meta_sb["gval2"]
        growl_sb = meta_sb["growl"]

        if max_phase >= 1:
            spmm_phase(b3_d, gval2_sb,
                       [(ident_sb, lambda tt: zslot(tt, 2))], sink1)

        # phase 2: b1 = z1 + 2 L b2 - b3(=z3)
        st2 = {"t": None}

        def sink2(tt, ps):
            if tt % GRP == 0:
                b1st = stg.tile([P, GRP, F], BF16, tag="b1st")
                st2["t"] = b1st
            nc.scalar.copy(out=st2["t"][:, tt % GRP, :], in_=ps[:])
            if tt % GRP == GRP - 1:
                nc.sync.dma_start(dram_group(b1_d, tt - GRP + 1, GRP),
                                  st2["t"][:])

        if max_phase >= 2:
            spmm_phase(b2_d, gval2_sb,
                       [(ident_sb, lambda tt: zslot(tt, 1)),
                        (nident_sb, lambda tt: zslot(tt, 3))], sink2)

        # phase 3: out = z0 + L b1 - b2(=z2 slot) + bias
        st3 = {"t": None}

        def sink3(tt, ps):
            if tt % GRP == 0:
                ost = ostg.tile([P, GRP, F], FP32, tag="ost")
                st3["t"] = ost
            nc.scalar.copy(out=st3["t"][:, tt % GRP, :], in_=ps[:])
            if tt % GRP == GRP - 1:
                nc.sync.dma_start(dram_group(out_d, tt - GRP + 1, GRP),
                                  st3["t"][:])

        if max_phase >= 3:
            spmm_phase(b1_d, gval1_sb,
                       [(ident_sb, lambda tt: zslot(tt, 0)),
                        (nident_sb, lambda tt: zslot(tt, 2)),
                        (ones_sb, lambda tt: biasm_sb[:])], sink3)

    nc.compile()
    return nc


def make_host_inputs(inputs, weight, bias, lap_vals, lap_rows, lap_cols):
    """Build per-core input maps + metadata. Returns (in_maps, key)."""
    gidx, growl, gval1, gval2, mm_chunk, mm_tile = _preprocess_lap(
        np.asarray(lap_rows), np.asarray(lap_cols),
        np.asarray(lap_vals, np.float32))

    w = np.asarray(weight, np.float32)          # [Fin, Kv, Kt, Fout]
    # wz[c=(t,f)][k, o]; column layout per cc-chunk of c: (cc*KV + k)*FOUT + o
    wzc = np.transpose(w, (2, 0, 1, 3)).reshape(C, KV * FOUT)
    wz = np.ascontiguousarray(
        wzc.reshape(2, P, KV * FOUT).transpose(1, 0, 2).reshape(P, 2 * KV * FOUT)
    ).astype(BF16_NP)

    ident = np.eye(P, dtype=np.float32).astype(BF16_NP)
    nident = (-np.eye(P, dtype=np.float32)).astype(BF16_NP)
    ones_col = np.zeros((P, P), np.float32)
    ones_col[0, :] = 1.0
    ones_col = ones_col.astype(BF16_NP)
    biasm = np.zeros((P, F), np.float32)
    biasm[0, :] = np.tile(np.asarray(bias, np.float32), BC)
    biasm = biasm.astype(BF16_NP)
    iota = np.ascontiguousarray(
        np.broadcast_to(np.arange(P, dtype=np.float32)[None, :], (P, P))
    ).astype(BF16_NP)

    common = {
        "wz": wz, "ident": ident, "nident": nident, "onescol": ones_col,
        "biasmat": biasm, "iota": iota, "gidx": gidx, "growl": growl,
        "gval1": gval1, "gval2": gval2,
    }
    x = np.asarray(inputs, np.float32)          # [B, V, T, Fin]
    in_maps = []
    for r in range(N_CORES):
        xc = x[BC * r:BC * (r + 1)].reshape(BC, V, C)        # [BC, V, 256]
        xt = np.ascontiguousarray(
            xc.transpose(0, 2, 1).reshape(BC, 2, P, V)).astype(BF16_NP)
        m = dict(common)
        m["xt"] = xt
        in_maps.append(m)
    return in_maps, (mm_chunk, mm_tile)


_CACHE = {}


def _get_program(key):
    if key not in _CACHE:
        _CACHE[key] = build_program(key[0], key[1])
    return _CACHE[key]


def kernel(inputs, weight, bias, lap_vals, lap_rows, lap_cols):
    in_maps, key = make_host_inputs(inputs, weight, bias, lap_vals,
                                    lap_rows, lap_cols)
    nc = _get_program(key)
    res = run_bass_kernel_spmd(nc, in_maps, list(range(N_CORES)))
    outs = []
    for r in range(N_CORES):
        o = res.results[r]["out"]                # [V, BC*FOUT]
        outs.append(o.reshape(V, BC, FOUT).transpose(1, 0, 2))
    out = np.concatenate(outs, axis=0)
    return np.ascontiguousarray(out.astype(np.float32))


def time_kernel(inputs_dict, iters=3):
    """Wall-clock repeated executions of the cached program (ns per run)."""
    import time

    in_maps, key = make_host_inputs(**inputs_dict)
    nc = _get_program(key)
    times = []
    for _ in range(iters):
        t0 = time.perf_counter()
        run_bass_kernel_spmd(nc, in_maps, list(range(N_CORES)))
        times.append(time.perf_counter() - t0)
    return min(times) * 1e9
